# revision 19
# baseline (speedup 1.0000x reference)
"""BinaryConv2D forward on 8 Trainium2 NeuronCores.

out = conv2d_same(inputs, sign(clip(kernel)))   (NHWC, HWIO, 3x3, stride 1)

Sharding: data-parallel over batch (32 images -> 4 per core); the 3x3x256x256
kernel is replicated (forward only, no gradient collective needed).

Per-core kernel strategy (v2: the PE runs conv matmuls ONLY):
  - weights: two f32 HWDGE loads (one per cout half, parallel on the
    SP/Act queues) feed two Activation sign ops emitting fp8e4
    [cin, pair, cout] stationary tiles (+-1 is exact in fp8), one tile
    per cout half so oc0's matmuls never wait on oc1's sign.
  - fp8 DoubleRow matmuls: one matmul contracts all 256 input channels at
    0.5 cycles/row. Precision from a two-level split x = hi + lo with
    hi = fp8(x), lo = fp8(x - hi), both streamed as accumulating passes
    (~bf16 accuracy at half the bf16 cycle count).
  - input path is PE-free: gpsimd SWDGE loads cast NHWC f32 -> bf16
    [112pix, 7blk, 256c] chunk tiles (both channel halves per DMA keeps
    the innermost run 512B, dodging the sub-512B DMA penalty; a 128KB
    descriptor carveout keeps the SWDGE FIFO from serializing);
    per-block XBAR DMA transposes (InstDmaTransposeAnt, 16x128 tiles,
    98ns) on the SP/Act HWDGE queues emit channel-major bf16 chunk
    tiles; Pool derives hi = fp8(x), lo = fp8(x - hi) per chunk into
    flat-padded [cin, cc, 58x57] images (57-wide rows share one zero
    column between row r's x=56 pad and row r+1's x=-1 pad). Tiles are
    per-chunk because DMA writes take whole-tile WAR deps against
    earlier readers — shared tiles would serialize the chunk pipeline.
  - conv as 9 shifted flat-window DoubleRow matmuls x {hi,lo} per psum
    block of 8 output rows (N=456, pad-row slices clipped at the image
    edges), accumulating 18 matmuls.
  - output path: DVE evicts PSUM -> bf16 [cout, pix]; after the oc's last
    evict, 25 XBAR transposes flip 128-pixel blocks back to pixel-major
    (emitted only after all evicts so the whole-tile WAR never stalls an
    evict), DVE upcasts bf16 -> f32, natural-layout DMA stores. All of it
    overlaps the next oc's conv. The very last oc instead uses v1's PE
    transpose path (PE is free at the tail) with fine-grained per-group
    stores, keeping the post-conv drain short.
  - warmup matmuls at t=0 keep the PE p-state ramp clock running while
    image 0 loads; image 0's hi/lo runs on DVE (chunks 0-1) and Pool
    (chunks 2-3); steady-state images prefetch on Pool/SP/Act entirely
    under the previous image's conv.

Cost-model (CoreSim) lineage: 450.4 us (bf16 2-pass) -> 141.8 us (v1:
fp8 DR + PE transposes) -> this rewrite (PE-only-conv + XBAR transposes).
"""

import numpy as np

P = 128
H = 56
W = 56
C = 256
XW = W + 2                   # padded row count (58: rows y=-1..56)
RW = W + 1                   # flat row stride: one shared zero col per row
FL = XW * RW                 # flat padded image length (3306)
FT = 3312                    # fp8 tile free size (junk pad to %16)
NCORES = 8
NTOT = 32
NI = NTOT // NCORES          # images per core
NPIX = H * W                 # 3136
RB = 8                       # output rows per psum block
NT = H // RB                 # 7 psum blocks
TB = 112                     # pixels per transpose block (= 2 rows)
NBLK = NPIX // TB            # 28 blocks exactly
NCH = 4                      # load chunks per image (7 blocks each)
CB = 7 * TB                  # pixels per chunk (784)
OB = 128                     # output transpose block (XBAR needs %128)
NOB = 25                     # ceil(3136/128) output blocks (last is 64 real)
OPIX = OB * NOB              # 3200 padded output pixels

_cache = {}


def _build_bass(ni=NI, loops=1, warm=135):
    import concourse.bacc as bacc
    import concourse.mybir as mybir
    import concourse.tile as tile
    from concourse.masks import make_identity
    from contextlib import ExitStack

    f32 = mybir.dt.float32
    bf16 = mybir.dt.bfloat16
    fp8 = mybir.dt.float8e4
    DR = mybir.MatmulPerfMode.DoubleRow

    nc = bacc.Bacc()
    x = nc.dram_tensor("x", [ni, NPIX, C], f32, kind="ExternalInput")
    w = nc.dram_tensor("w", [3, 3, C, C], f32, kind="ExternalInput")
    y = nc.dram_tensor("y", [ni, NPIX, C], f32, kind="ExternalOutput")

    with ExitStack() as ctx:
        tc = ctx.enter_context(tile.TileContext(nc))
        const = ctx.enter_context(tc.tile_pool(name="const", bufs=1))
        wpool = ctx.enter_context(tc.tile_pool(name="wpool", bufs=1))
        wstage = ctx.enter_context(tc.tile_pool(name="wstage", bufs=1))
        hinp = ctx.enter_context(tc.tile_pool(name="hinp", bufs=1))
        xbp = ctx.enter_context(tc.tile_pool(name="xbp", bufs=1))
        padp = ctx.enter_context(tc.tile_pool(name="padp", bufs=2))
        ocp = ctx.enter_context(tc.tile_pool(name="ocp", bufs=2))
        otp = ctx.enter_context(tc.tile_pool(name="otp", bufs=2))
        onp = ctx.enter_context(tc.tile_pool(name="onp", bufs=2))
        onf = ctx.enter_context(tc.tile_pool(name="onf", bufs=2))
        psc = ctx.enter_context(tc.tile_pool(name="psc", bufs=3, space="PSUM"))
        psf = ctx.enter_context(tc.tile_pool(name="psf", bufs=2, space="PSUM"))
        pst = ctx.enter_context(tc.tile_pool(name="pst", bufs=3, space="PSUM"))

        identb = const.tile([P, P], bf16)
        make_identity(nc, identb)
        identf = const.tile([P, P], f32)
        nc.vector.tensor_copy(out=identf, in_=identb)

        dmaq = [nc.sync, nc.scalar]

        # ---- binarized weight tiles: sign(w) as fp8 [cin, pair, cout],
        # one shared f32 stage tile (reloaded per cout half) + fp8 sign
        # tiles per half so oc0's matmuls never wait on oc1's sign ----
        wsgn = [wpool.tile([P, 9, 2, P], fp8, name=f"wsgn{o}")
                for o in range(2)]

        def _load_w(oc, queue=None):
            wt = wstage.tile([P, 9, 2, P], f32, name="wst")
            dmaq[oc if queue is None else queue].dma_start(
                out=wt,
                in_=w[:, :, :, P * oc : P * (oc + 1)].rearrange(
                    "ky kx (cc p) o -> p (ky kx) cc o", p=P
                ),
            )
            return wt

        def _sign(oc, wt):
            nc.scalar.sign(out=wsgn[oc], in_=wt)

        # HAM warmup: dummy matmuls keep the PE busy from t~0 while the
        # first image loads, so the p-state ramp reaches full clock before
        # the first conv matmuls arrive. Results are never read.
        wrm = psc.tile([P, RB, RW], f32, name="ps")
        for _ in range(warm):
            nc.tensor.matmul(
                wrm[:, :2, :], lhsT=identb, rhs=identb[:, : 2 * RW],
                start=True, stop=True,
            )

        def _alloc_image(img):
            st = {"img": img}
            st["xnat"] = [hinp.tile([TB, 7, C], f32, name=f"xnat{q}")
                          for q in range(NCH)]
            st["hin"] = [hinp.tile([TB, 7, C], bf16, name=f"hin{q}")
                         for q in range(NCH)]
            st["xpb"] = [xbp.tile([P, 2, CB], bf16, name=f"xpb{q}")
                         for q in range(NCH)]
            st["xph"] = padp.tile([P, 2, FT], fp8, name="xph")
            st["xpl"] = padp.tile([P, 2, FT], fp8, name="xpl")
            # zero the SAME-padding borders (rows y=-1,56 and cols x=-1,56)
            # and the junk edge cells some shifted windows read
            for xp8 in (st["xph"], st["xpl"]):
                nc.vector.memset(xp8[:, :, 0:1], 0.0)
                nc.vector.memset(xp8[:, :, 1 + FL : FT], 0.0)
                xv = xp8[:, :, 1 : 1 + FL].rearrange(
                    "p j (r c) -> p j r c", c=RW
                )
                nc.vector.memset(xv[:, :, 0, :], 0.0)
                nc.vector.memset(xv[:, :, XW - 1, :], 0.0)
                nc.vector.memset(xv[:, :, 1 : XW - 1, 0], 0.0)
            return st

        def _load_chunk(st, q, queue=None):
            # HWDGE f32 load (1KB innermost, full DMA rate); Pool casts to
            # bf16. SWDGE casting loads would halve the traffic but the
            # SWDGE FIFO's sem recycling couples loads to reader progress
            # and serializes the whole prep pipeline.
            dmaq[q % 2 if queue is None else queue].dma_start(
                out=st["xnat"][q],
                in_=x[st["img"], :, :].rearrange(
                    "(q b p) c -> q p b c", p=TB, q=NCH
                )[q],
            )

        def _cast_chunk(st, q):
            nc.gpsimd.tensor_copy(out=st["hin"][q], in_=st["xnat"][q])

        def _emit_xbar(st, q, queue=None):
            # XBAR-transpose chunk q's blocks into channel-major bf16.
            # All 14 land on ONE queue: a tile written from two queues
            # serializes on cross-queue sync, one queue pipelines at 98ns.
            eng = dmaq[q % 2 if queue is None else queue]
            for b in range(7):
                for cc in range(2):
                    eng.dma_start(
                        out=st["xpb"][q][:, cc, TB * b : TB * (b + 1)],
                        in_=st["hin"][q][:, b, P * cc : P * (cc + 1)],
                        transpose=True,
                    )

        def _emit_hilo(st, q, eng):
            # derive hi = fp8(x), lo = fp8(x - hi) for chunk q (14 image
            # rows) into the flat-padded fp8 images
            r0 = 14 * q + 1
            for cc in range(2):
                bv = st["xpb"][q][:, cc, :].rearrange(
                    "p (r c) -> p r c", c=W
                )

                def _dst(xp8):
                    return xp8[:, cc, 1 : 1 + FL].rearrange(
                        "p (r c) -> p r c", c=RW
                    )[:, r0 : r0 + 14, 1 : 1 + W]

                eng.tensor_copy(out=_dst(st["xph"]), in_=bv)
                eng.tensor_sub(out=_dst(st["xpl"]), in0=bv,
                               in1=_dst(st["xph"]))

        def _emit_pe_input(st, q, ev):
            # image-0 fast path: the PE is idle at startup, so transpose the
            # f32 chunks directly on the PE (2 cyc/row, no cast needed),
            # evict psum -> bf16 on DVE/Act, per-block hi/lo on Pool. This
            # skips the cast + XBAR latency hops on the critical path.
            for b7 in range(7):
                b = 7 * q + b7
                for cc in range(2):
                    pt = pst.tile([P, TB], f32, name="pstt")
                    nc.tensor.transpose(
                        pt, st["xnat"][q][:, b7, P * cc : P * (cc + 1)],
                        identf[:TB, :TB],
                    )
                    bb = st["xpb"][q][:, cc, TB * b7 : TB * (b7 + 1)]
                    if ev is nc.scalar:
                        nc.scalar.copy(out=bb, in_=pt)
                    else:
                        ev.tensor_copy(out=bb, in_=pt)
                    bv = bb.rearrange("p (two xx) -> p two xx", two=2)

                    def _dst(xp8):
                        return xp8[:, cc, 1 : 1 + FL].rearrange(
                            "p (r c) -> p r c", c=RW
                        )[:, 2 * b + 1 : 2 * b + 3, 1 : 1 + W]

                    nc.gpsimd.tensor_copy(out=_dst(st["xph"]), in_=bv)
                    nc.gpsimd.tensor_sub(out=_dst(st["xpl"]), in0=bv,
                                         in1=_dst(st["xph"]))

        def _emit_out_xbar(img, oc, ocmp):
            # flip the finished oc image back to pixel-major, upcast, store.
            # Runs entirely on SP/Act/DVE, overlapping the next oc's conv.
            ot = otp.tile([P, NOB, P], bf16, name="ot")
            onat = onp.tile([P, NOB, P], f32, name="onat")
            for j in range(NOB):
                dmaq[oc].dma_start(
                    out=ot[:, j, :],
                    in_=ocmp[:, OB * j : OB * (j + 1)],
                    transpose=True,
                )
            for bi, (j0, j1) in enumerate(((0, 12), (12, NOB))):
                nc.scalar.copy(out=onat[:, j0:j1], in_=ot[:, j0:j1])
                jr = min(j1, NOB - 1)
                dmaq[(bi + oc) % 2].dma_start(
                    out=y[
                        img, OB * j0 : OB * jr, P * oc : P * (oc + 1)
                    ].rearrange("(b p) c -> p b c", p=OB),
                    in_=onat[:, j0:jr],
                )
            dmaq[oc % 2].dma_start(
                out=y[img, OB * (NOB - 1) : NPIX, P * oc : P * (oc + 1)
                      ].rearrange("(b p) c -> p b c", p=64),
                in_=onat[:64, NOB - 1],
            )

        def _emit_group_pe(img, oc, ocmp, t):
            # v1-style PE transpose path for the very last oc: group t's 4
            # TB-blocks (448 pixels) go psum->sbuf->store right away, so
            # the post-conv drain is one small group
            pt = psf.tile([P, 4, P], bf16, name="ptf")
            onatf = onf.tile([P, 4, P], f32, name="onatf")
            for bi in range(4):
                b = 4 * t + bi
                nc.tensor.transpose(
                    pt[:TB, bi, :], ocmp[:, TB * b : TB * (b + 1)], identb
                )
            nc.scalar.copy(out=onatf[:TB, :], in_=pt[:TB, :])
            for qi in range(2):
                dmaq[qi].dma_start(
                    out=y[
                        img,
                        TB * (4 * t + 2 * qi) : TB * (4 * t + 2 * qi + 2),
                        P * oc : P * (oc + 1),
                    ].rearrange("(b p) c -> p b c", p=TB),
                    in_=onatf[:TB, 2 * qi : 2 * qi + 2],
                )

        def _conv_image(st, nxt, prep=None):
            # ---- conv: 18 accumulating DoubleRow matmuls per psum block
            # (hi/lo passes x 9 taps, all 256 cin per matmul). All other
            # work rides on DVE/Pool/SP/Act and never touches the PE queue
            # (except image 0's input transposes and the last oc's tail
            # transposes, when the PE has slack). ----
            img = st["img"]
            combos = [
                (st["xph"], ky, kx) for ky in (1, 0, 2) for kx in range(3)
            ] + [
                (st["xpl"], ky, kx) for ky in (1, 0, 2) for kx in range(3)
            ]
            n_c = len(combos)

            for oc in range(2):
                fine = nxt is None and oc == 1
                ocmp = ocp.tile([P, OPIX], bf16, name="ocmp")
                if not fine:
                    nc.vector.memset(ocmp[:, NPIX:OPIX], 0.0)
                for t in range(NT):
                    if prep is not None and oc == 0 and t in prep:
                        q, ev = prep[t]
                        _emit_pe_input(st, q, ev)
                    ps = psc.tile([P, RB, RW], f32, name="ps")
                    for ci, (src8, ky, kx) in enumerate(combos):
                        dy = ky - 1
                        fs = (RB * t + dy + 1) * RW + kx
                        # skip the zero pad-row slice of the window for the
                        # edge taps (ci==0 is dy=0, so the start flag still
                        # clears the full region)
                        r0 = 1 if (t == 0 and dy < 0) else 0
                        r1 = RB - (1 if (t == NT - 1 and dy > 0) else 0)
                        nc.tensor.matmul(
                            ps[:, r0:r1, :],
                            lhsT=wsgn[oc][:, 3 * ky + kx, :, :],
                            rhs=src8[:, :, fs + r0 * RW : fs + r1 * RW],
                            start=(ci == 0),
                            stop=(ci == n_c - 1),
                            perf_mode=DR,
                        )
                        # spread next-image prep between this group's
                        # matmuls (non-PE queues, dependency-time order)
                        if nxt is not None and ci == 4:
                            if oc == 0:
                                if t in (0, 1, 2, 3):
                                    _load_chunk(nxt, t)
                                if t in (1, 2, 3, 4):
                                    _cast_chunk(nxt, t - 1)
                                    _emit_xbar(nxt, t - 1)
                            elif t in (0, 1, 2, 3):
                                _emit_hilo(nxt, t, nc.gpsimd)
                    # evict this group's rows to the bf16 compact image
                    nc.vector.tensor_copy(
                        out=ocmp[:, RB * W * t : RB * W * (t + 1)],
                        in_=ps[:, :, 1 : 1 + W],
                    )
                    if fine:
                        _emit_group_pe(img, oc, ocmp, t)
                if not fine:
                    _emit_out_xbar(img, oc, ocmp)

        def _images():
            # image 0 startup: chunk-0's load -> XBAR -> hi/lo chain is the
            # critical path to the first conv matmul. Weights ride the
            # HWDGE queues (parallel to the SWDGE chunk loads on Pool);
            # image-0 hi/lo runs on DVE (chunks 0-1) and Pool (2-3).
            st = _alloc_image(0)
            _load_chunk(st, 0, queue=0)
            wt0 = _load_w(0, queue=1)
            _sign(0, wt0)
            wt1 = _load_w(1, queue=0)
            _load_chunk(st, 1, queue=1)
            _sign(1, wt1)
            _load_chunk(st, 2, queue=0)
            _load_chunk(st, 3, queue=1)
            prep0 = {0: (0, nc.vector), 1: (1, nc.vector),
                     2: (2, nc.scalar), 4: (3, nc.scalar)}
            for img in range(ni):
                nxt = _alloc_image(img + 1) if img + 1 < ni else None
                _conv_image(st, nxt, prep=prep0 if img == 0 else None)
                st = nxt

        if loops == 1:
            _images()
        else:
            with tc.For_i(0, loops, 1):
                _images()
    nc.compile()
    return nc


def get_bass(ni=NI, loops=1):
    key = (ni, loops)
    if key not in _cache:
        _cache[key] = _build_bass(ni, loops)
    return _cache[key]


def run(inputs, kernel, trace=False, **kw):
    from concourse.bass_utils import run_bass_kernel_spmd

    nc = get_bass()
    xs = np.ascontiguousarray(inputs, dtype=np.float32).reshape(NTOT, NPIX, C)
    wf = np.ascontiguousarray(kernel, dtype=np.float32)
    in_maps = [
        {"x": xs[i * NI : (i + 1) * NI], "w": wf} for i in range(NCORES)
    ]
    res = run_bass_kernel_spmd(nc, in_maps, core_ids=list(range(NCORES)),
                               trace=trace, **kw)
    out = np.concatenate([r["y"] for r in res.results], axis=0)
    return out.reshape(NTOT, H, W, C), res


def kernel(**inputs):
    out, _ = run(inputs["inputs"], inputs["kernel"])
    return out


# revision 20
# speedup vs baseline: 1.0777x; 1.0777x over previous
"""BinaryConv2D forward on 8 Trainium2 NeuronCores.

out = conv2d_same(inputs, sign(clip(kernel)))   (NHWC, HWIO, 3x3, stride 1)

Sharding: data-parallel over batch (32 images -> 4 per core); the 3x3x256x256
kernel is replicated (forward only, no gradient collective needed).

Per-core kernel strategy (v2: the PE runs conv matmuls ONLY):
  - weights: two f32 HWDGE loads (one per cout half, parallel on the
    SP/Act queues) feed two Activation sign ops emitting fp8e4
    [cin, pair, cout] stationary tiles (+-1 is exact in fp8), one tile
    per cout half so oc0's matmuls never wait on oc1's sign.
  - fp8 DoubleRow matmuls: one matmul contracts all 256 input channels at
    0.5 cycles/row. Precision from a two-level split x = hi + lo with
    hi = fp8(x), lo = fp8(x - hi), both streamed as accumulating passes
    (~bf16 accuracy at half the bf16 cycle count).
  - input path is PE-free: gpsimd SWDGE loads cast NHWC f32 -> bf16
    [112pix, 7blk, 256c] chunk tiles (both channel halves per DMA keeps
    the innermost run 512B, dodging the sub-512B DMA penalty; a 128KB
    descriptor carveout keeps the SWDGE FIFO from serializing);
    per-block XBAR DMA transposes (InstDmaTransposeAnt, 16x128 tiles,
    98ns) on the SP/Act HWDGE queues emit channel-major bf16 chunk
    tiles; Pool derives hi = fp8(x), lo = fp8(x - hi) per chunk into
    flat-padded [cin, cc, 58x57] images (57-wide rows share one zero
    column between row r's x=56 pad and row r+1's x=-1 pad). Tiles are
    per-chunk because DMA writes take whole-tile WAR deps against
    earlier readers — shared tiles would serialize the chunk pipeline.
  - conv as 9 shifted flat-window DoubleRow matmuls x {hi,lo} per psum
    block of 8 output rows (N=456, pad-row slices clipped at the image
    edges), accumulating 18 matmuls.
  - output path: DVE evicts PSUM -> bf16 [cout, pix]; after the oc's last
    evict, 25 XBAR transposes flip 128-pixel blocks back to pixel-major
    (emitted only after all evicts so the whole-tile WAR never stalls an
    evict), DVE upcasts bf16 -> f32, natural-layout DMA stores. All of it
    overlaps the next oc's conv. The very last oc instead uses v1's PE
    transpose path (PE is free at the tail) with fine-grained per-group
    stores, keeping the post-conv drain short.
  - warmup matmuls at t=0 keep the PE p-state ramp clock running while
    image 0 loads; image 0's hi/lo runs on DVE (chunks 0-1) and Pool
    (chunks 2-3); steady-state images prefetch on Pool/SP/Act entirely
    under the previous image's conv.

Cost-model (CoreSim) lineage: 450.4 us (bf16 2-pass) -> 141.8 us (v1:
fp8 DR + PE transposes) -> this rewrite (PE-only-conv + XBAR transposes).
"""

import numpy as np

P = 128
H = 56
W = 56
C = 256
XW = W + 2                   # padded row count (58: rows y=-1..56)
RW = W + 1                   # flat row stride: one shared zero col per row
FL = XW * RW                 # flat padded image length (3306)
FT = 3312                    # fp8 tile free size (junk pad to %16)
NCORES = 8
NTOT = 32
NI = NTOT // NCORES          # images per core
NPIX = H * W                 # 3136
RB = 8                       # output rows per psum block
NT = H // RB                 # 7 psum blocks
TB = 112                     # pixels per transpose block (= 2 rows)
NBLK = NPIX // TB            # 28 blocks exactly
NCH = 4                      # load chunks per image (7 blocks each)
CB = 7 * TB                  # pixels per chunk (784)
OB = 128                     # output transpose block (XBAR needs %128)
NOB = 25                     # ceil(3136/128) output blocks (last is 64 real)
OPIX = OB * NOB              # 3200 padded output pixels

_cache = {}


def _build_bass(ni=NI, loops=1, warm=135):
    import concourse.bacc as bacc
    import concourse.mybir as mybir
    import concourse.tile as tile
    from concourse.masks import make_identity
    from contextlib import ExitStack

    f32 = mybir.dt.float32
    bf16 = mybir.dt.bfloat16
    fp8 = mybir.dt.float8e4
    DR = mybir.MatmulPerfMode.DoubleRow

    nc = bacc.Bacc()
    x = nc.dram_tensor("x", [ni, NPIX, C], f32, kind="ExternalInput")
    w = nc.dram_tensor("w", [3, 3, C, C], f32, kind="ExternalInput")
    y = nc.dram_tensor("y", [ni, NPIX, C], f32, kind="ExternalOutput")

    with ExitStack() as ctx:
        tc = ctx.enter_context(tile.TileContext(nc))
        const = ctx.enter_context(tc.tile_pool(name="const", bufs=1))
        wpool = ctx.enter_context(tc.tile_pool(name="wpool", bufs=1))
        wstage = ctx.enter_context(tc.tile_pool(name="wstage", bufs=1))
        hinp = ctx.enter_context(tc.tile_pool(name="hinp", bufs=1))
        xbp = ctx.enter_context(tc.tile_pool(name="xbp", bufs=1))
        padp = ctx.enter_context(tc.tile_pool(name="padp", bufs=2))
        ocp = ctx.enter_context(tc.tile_pool(name="ocp", bufs=2))
        otp = ctx.enter_context(tc.tile_pool(name="otp", bufs=2))
        onp = ctx.enter_context(tc.tile_pool(name="onp", bufs=2))
        onf = ctx.enter_context(tc.tile_pool(name="onf", bufs=2))
        psc = ctx.enter_context(tc.tile_pool(name="psc", bufs=3, space="PSUM"))
        psf = ctx.enter_context(tc.tile_pool(name="psf", bufs=2, space="PSUM"))
        pst = ctx.enter_context(tc.tile_pool(name="pst", bufs=3, space="PSUM"))

        identb = const.tile([P, P], bf16)
        make_identity(nc, identb)
        identf = const.tile([P, P], f32)
        nc.vector.tensor_copy(out=identf, in_=identb)

        dmaq = [nc.sync, nc.scalar]

        # ---- binarized weight tiles: sign(w) as fp8 [cin, pair, cout],
        # one shared f32 stage tile (reloaded per cout half) + fp8 sign
        # tiles per half so oc0's matmuls never wait on oc1's sign ----
        wsgn = [wpool.tile([P, 9, 2, P], fp8, name=f"wsgn{o}")
                for o in range(2)]

        def _load_w(oc, queue=None):
            wt = wstage.tile([P, 9, 2, P], f32, name="wst")
            dmaq[oc if queue is None else queue].dma_start(
                out=wt,
                in_=w[:, :, :, P * oc : P * (oc + 1)].rearrange(
                    "ky kx (cc p) o -> p (ky kx) cc o", p=P
                ),
            )
            return wt

        def _sign(oc, wt):
            nc.scalar.sign(out=wsgn[oc], in_=wt)

        # HAM warmup: dummy matmuls keep the PE busy from t~0 while the
        # first image loads, so the p-state ramp reaches full clock before
        # the first conv matmuls arrive. Results are never read.
        wrm = psc.tile([P, RB, RW], f32, name="ps")
        for _ in range(warm):
            nc.tensor.matmul(
                wrm[:, :2, :], lhsT=identb, rhs=identb[:, : 2 * RW],
                start=True, stop=True,
            )

        def _alloc_image(img):
            st = {"img": img}
            st["xnat"] = [hinp.tile([TB, 7, C], f32, name=f"xnat{q}")
                          for q in range(NCH)]
            st["hin"] = [hinp.tile([TB, 7, C], bf16, name=f"hin{q}")
                         for q in range(NCH)]
            st["xpb"] = [xbp.tile([P, 2, CB], bf16, name=f"xpb{q}")
                         for q in range(NCH)]
            st["xph"] = padp.tile([P, 2, FT], fp8, name="xph")
            st["xpl"] = padp.tile([P, 2, FT], fp8, name="xpl")
            # zero the SAME-padding borders (rows y=-1,56 and cols x=-1,56)
            # and the junk edge cells some shifted windows read
            for xp8 in (st["xph"], st["xpl"]):
                nc.vector.memset(xp8[:, :, 0:1], 0.0)
                nc.vector.memset(xp8[:, :, 1 + FL : FT], 0.0)
                xv = xp8[:, :, 1 : 1 + FL].rearrange(
                    "p j (r c) -> p j r c", c=RW
                )
                nc.vector.memset(xv[:, :, 0, :], 0.0)
                nc.vector.memset(xv[:, :, XW - 1, :], 0.0)
                nc.vector.memset(xv[:, :, 1 : XW - 1, 0], 0.0)
            return st

        def _load_chunk(st, q, queue=None):
            # HWDGE f32 load (1KB innermost, full DMA rate); Pool casts to
            # bf16. SWDGE casting loads would halve the traffic but the
            # SWDGE FIFO's sem recycling couples loads to reader progress
            # and serializes the whole prep pipeline.
            dmaq[q % 2 if queue is None else queue].dma_start(
                out=st["xnat"][q],
                in_=x[st["img"], :, :].rearrange(
                    "(q b p) c -> q p b c", p=TB, q=NCH
                )[q],
            )

        def _cast_chunk(st, q):
            nc.gpsimd.tensor_copy(out=st["hin"][q], in_=st["xnat"][q])

        def _emit_xbar(st, q, queue=None):
            # XBAR-transpose chunk q's blocks into channel-major bf16.
            # All 14 land on ONE queue: a tile written from two queues
            # serializes on cross-queue sync, one queue pipelines at 98ns.
            eng = dmaq[q % 2 if queue is None else queue]
            for b in range(7):
                for cc in range(2):
                    eng.dma_start(
                        out=st["xpb"][q][:, cc, TB * b : TB * (b + 1)],
                        in_=st["hin"][q][:, b, P * cc : P * (cc + 1)],
                        transpose=True,
                    )

        def _emit_hilo(st, q, eng):
            # derive hi = fp8(x), lo = fp8(x - hi) for chunk q (14 image
            # rows) into the flat-padded fp8 images
            r0 = 14 * q + 1
            for cc in range(2):
                bv = st["xpb"][q][:, cc, :].rearrange(
                    "p (r c) -> p r c", c=W
                )

                def _dst(xp8):
                    return xp8[:, cc, 1 : 1 + FL].rearrange(
                        "p (r c) -> p r c", c=RW
                    )[:, r0 : r0 + 14, 1 : 1 + W]

                eng.tensor_copy(out=_dst(st["xph"]), in_=bv)
                eng.tensor_sub(out=_dst(st["xpl"]), in0=bv,
                               in1=_dst(st["xph"]))

        def _emit_pe_input(st, q, ev):
            # image-0 fast path: the PE is idle at startup, so transpose the
            # f32 chunks directly on the PE (2 cyc/row, no cast needed),
            # evict psum -> bf16 on DVE/Act, per-block hi/lo on Pool. This
            # skips the cast + XBAR latency hops on the critical path.
            for b7 in range(7):
                b = 7 * q + b7
                for cc in range(2):
                    pt = pst.tile([P, TB], f32, name="pstt")
                    nc.tensor.transpose(
                        pt, st["xnat"][q][:, b7, P * cc : P * (cc + 1)],
                        identf[:TB, :TB],
                    )
                    bb = st["xpb"][q][:, cc, TB * b7 : TB * (b7 + 1)]
                    if ev is nc.scalar:
                        nc.scalar.copy(out=bb, in_=pt)
                    else:
                        ev.tensor_copy(out=bb, in_=pt)
                    bv = bb.rearrange("p (two xx) -> p two xx", two=2)

                    def _dst(xp8):
                        return xp8[:, cc, 1 : 1 + FL].rearrange(
                            "p (r c) -> p r c", c=RW
                        )[:, 2 * b + 1 : 2 * b + 3, 1 : 1 + W]

                    nc.gpsimd.tensor_copy(out=_dst(st["xph"]), in_=bv)
                    nc.gpsimd.tensor_sub(out=_dst(st["xpl"]), in0=bv,
                                         in1=_dst(st["xph"]))

        def _emit_out_xbar(img, oc, ocmp):
            # flip the finished oc image back to pixel-major on one HWDGE
            # queue; the upcast + store flush happens half an image later
            # (via _flush_out) so nothing ever waits on it
            ot = otp.tile([P, NOB, P], bf16, name="ot")
            onat = onp.tile([P, NOB, P], f32, name="onat")
            for j in range(NOB):
                dmaq[oc].dma_start(
                    out=ot[:, j, :],
                    in_=ocmp[:, OB * j : OB * (j + 1)],
                    transpose=True,
                )
            return (img, oc, ot, onat)

        def _flush_out(fl):
            # bf16 -> f32 upcast (Pool) + natural-layout stores, emitted
            # long after the out-xbars completed
            img, oc, ot, onat = fl
            for bi, (j0, j1) in enumerate(((0, 12), (12, NOB))):
                nc.gpsimd.tensor_copy(out=onat[:, j0:j1], in_=ot[:, j0:j1])
                jr = min(j1, NOB - 1)
                dmaq[(bi + oc) % 2].dma_start(
                    out=y[
                        img, OB * j0 : OB * jr, P * oc : P * (oc + 1)
                    ].rearrange("(b p) c -> p b c", p=OB),
                    in_=onat[:, j0:jr],
                )
            dmaq[oc % 2].dma_start(
                out=y[img, OB * (NOB - 1) : NPIX, P * oc : P * (oc + 1)
                      ].rearrange("(b p) c -> p b c", p=64),
                in_=onat[:64, NOB - 1],
            )

        def _emit_group_pe(img, oc, ocmp, t):
            # v1-style PE transpose path for the very last oc: group t's 4
            # TB-blocks (448 pixels) go psum->sbuf->store right away, so
            # the post-conv drain is one small group
            pt = psf.tile([P, 4, P], bf16, name="ptf")
            onatf = onf.tile([P, 4, P], f32, name="onatf")
            for bi in range(4):
                b = 4 * t + bi
                nc.tensor.transpose(
                    pt[:TB, bi, :], ocmp[:, TB * b : TB * (b + 1)], identb
                )
            nc.scalar.copy(out=onatf[:TB, :], in_=pt[:TB, :])
            for qi in range(2):
                dmaq[qi].dma_start(
                    out=y[
                        img,
                        TB * (4 * t + 2 * qi) : TB * (4 * t + 2 * qi + 2),
                        P * oc : P * (oc + 1),
                    ].rearrange("(b p) c -> p b c", p=TB),
                    in_=onatf[:TB, 2 * qi : 2 * qi + 2],
                )

        pending_flush = []

        def _conv_image(st, nxt, prep=None):
            # ---- conv: 18 accumulating DoubleRow matmuls per psum block
            # (hi/lo passes x 9 taps, all 256 cin per matmul). All other
            # work rides on DVE/Pool/SP/Act and never touches the PE queue
            # (except image 0's input transposes and the last oc's tail
            # transposes, when the PE has slack). ----
            img = st["img"]
            combos = [
                (st["xph"], ky, kx) for ky in (1, 0, 2) for kx in range(3)
            ] + [
                (st["xpl"], ky, kx) for ky in (1, 0, 2) for kx in range(3)
            ]
            n_c = len(combos)

            for oc in range(2):
                fine = nxt is None and oc == 1
                ocmp = ocp.tile([P, OPIX], bf16, name="ocmp")
                if not fine:
                    nc.vector.memset(ocmp[:, NPIX:OPIX], 0.0)
                for t in range(NT):
                    if prep is not None and oc == 0 and t in prep:
                        q, ev = prep[t]
                        _emit_pe_input(st, q, ev)
                    ps = psc.tile([P, RB, RW], f32, name="ps")
                    for ci, (src8, ky, kx) in enumerate(combos):
                        dy = ky - 1
                        fs = (RB * t + dy + 1) * RW + kx
                        # skip the zero pad-row slice of the window for the
                        # edge taps (ci==0 is dy=0, so the start flag still
                        # clears the full region)
                        r0 = 1 if (t == 0 and dy < 0) else 0
                        r1 = RB - (1 if (t == NT - 1 and dy > 0) else 0)
                        nc.tensor.matmul(
                            ps[:, r0:r1, :],
                            lhsT=wsgn[oc][:, 3 * ky + kx, :, :],
                            rhs=src8[:, :, fs + r0 * RW : fs + r1 * RW],
                            start=(ci == 0),
                            stop=(ci == n_c - 1),
                            perf_mode=DR,
                        )
                        # spread next-image prep between this group's
                        # matmuls (non-PE queues, dependency-time order)
                        if nxt is not None and ci == 4:
                            if oc == 0:
                                if t in (0, 1, 2, 3):
                                    _load_chunk(nxt, t)
                                if t in (1, 2, 3, 4):
                                    _cast_chunk(nxt, t - 1)
                                    _emit_xbar(nxt, t - 1)
                            elif t in (0, 1, 2, 3):
                                _emit_hilo(nxt, t, nc.gpsimd)
                        if ci == 4 and t == 5 - oc and pending_flush:
                            _flush_out(pending_flush.pop(0))
                    # evict this group's rows to the bf16 compact image
                    nc.vector.tensor_copy(
                        out=ocmp[:, RB * W * t : RB * W * (t + 1)],
                        in_=ps[:, :, 1 : 1 + W],
                    )
                    if fine:
                        _emit_group_pe(img, oc, ocmp, t)
                if not fine:
                    pending_flush.append(_emit_out_xbar(img, oc, ocmp))

        def _images():
            # image 0 startup: chunk-0's load -> XBAR -> hi/lo chain is the
            # critical path to the first conv matmul. Weights ride the
            # HWDGE queues (parallel to the SWDGE chunk loads on Pool);
            # image-0 hi/lo runs on DVE (chunks 0-1) and Pool (2-3).
            st = _alloc_image(0)
            _load_chunk(st, 0, queue=0)
            wt0 = _load_w(0, queue=1)
            _sign(0, wt0)
            wt1 = _load_w(1, queue=0)
            _load_chunk(st, 1, queue=1)
            _sign(1, wt1)
            _load_chunk(st, 2, queue=0)
            _load_chunk(st, 3, queue=1)
            prep0 = {0: (0, nc.vector), 1: (1, nc.vector),
                     2: (2, nc.scalar), 4: (3, nc.scalar)}
            for img in range(ni):
                nxt = _alloc_image(img + 1) if img + 1 < ni else None
                _conv_image(st, nxt, prep=prep0 if img == 0 else None)
                st = nxt
            for fl in pending_flush:
                _flush_out(fl)

        if loops == 1:
            _images()
        else:
            with tc.For_i(0, loops, 1):
                _images()
    nc.compile()
    return nc


def get_bass(ni=NI, loops=1):
    key = (ni, loops)
    if key not in _cache:
        _cache[key] = _build_bass(ni, loops)
    return _cache[key]


def run(inputs, kernel, trace=False, **kw):
    from concourse.bass_utils import run_bass_kernel_spmd

    nc = get_bass()
    xs = np.ascontiguousarray(inputs, dtype=np.float32).reshape(NTOT, NPIX, C)
    wf = np.ascontiguousarray(kernel, dtype=np.float32)
    in_maps = [
        {"x": xs[i * NI : (i + 1) * NI], "w": wf} for i in range(NCORES)
    ]
    res = run_bass_kernel_spmd(nc, in_maps, core_ids=list(range(NCORES)),
                               trace=trace, **kw)
    out = np.concatenate([r["y"] for r in res.results], axis=0)
    return out.reshape(NTOT, H, W, C), res


def kernel(**inputs):
    out, _ = run(inputs["inputs"], inputs["kernel"])
    return out
